# revision 27
# baseline (speedup 1.0000x reference)
"""Trainium2 Bass kernel for nn_EquivariantAttention (gnn_message_passing).

Sharding: nodes (and their 16 edge slots) are split across the 8 NeuronCores
(2560 padded nodes per core).  Node features `f` are replicated to every core
(2.5 MB) and the per-edge gather f[neighbor_idx] is done on-device with
indirect DMA.  Radial MLPs run on TensorE, per-edge tensor products on
VectorE, softmax denominators / attention-weighted sums via small mask
matmuls on TensorE (reducing over the 16 edge slots that live in the
partition dimension).

Self-contained: only needs numpy + the system toolchain at /opt/trn_rl_repo.
"""

import sys
from contextlib import ExitStack

import numpy as np

if "/opt/trn_rl_repo" not in sys.path:
    sys.path.insert(0, "/opt/trn_rl_repo")

import concourse.bacc as bacc
import concourse.mybir as mybir
import concourse.tile as tile
from concourse import bass
from concourse.masks import make_identity


# ---- custom DVE op: fused multiply + segmented running-sum ------------------
import dataclasses as _dc

from concourse import dve_spec as _DS
from concourse import dve_ops as _DO
from concourse.dve_uop import DveOpSpec as _DveOpSpec


def _ref_segsum(in0, in1, c0, c1, c2):
    x = in0.astype(np.float32) * in1.astype(np.float32)
    return np.cumsum(x, axis=-1)


def _lower_segsum(spec, ver):
    n_lanes, n_stages = _DS.N_LANES[ver], _DS.N_STAGES[ver]
    _DS._validate_body(spec, ver)
    spec2 = _DS._hoist_stream_invariant_ops(spec)
    scans = _DS._collect(spec2.body, _DS.Scan)
    latches = _DS._collect(spec2.body, _DS.Latch)
    p = _DS._build_placement(spec2, scans, n_stages, n_lanes)
    states = _DS._build_state_machine(spec2, scans, latches, p)
    sc = scans[0]
    d = p.node_stage[sc]
    step_ov = {d: _DS._Stage(_DS.AluOp.BYPASS, sc.expr)}
    seed, steady = states
    steady2 = _dc.replace(
        seed.__class__(**{**_dc.asdict(steady)}) if False else steady,
        trigger=(_DS.Trigger.SRC_TENSOR_DONE, _DS.Trigger.SUB_DIM_DONE,
                 _DS.Trigger.NONE),
        next=(0, 2, 0),
    )
    step = _dc.replace(
        steady,
        overrides=step_ov,
        trigger=(_DS.Trigger.SRC_TENSOR_DONE, _DS.Trigger.SUB_DIM_DONE,
                 _DS.Trigger.COUNT),
        next=(0, 2, 1),
        repeat=1,
    )
    uops = [_DS._assemble(s) for s in (seed, steady2, step)]
    for u in uops:
        u.validate(ver)
    return uops


@_dc.dataclass(frozen=True)
class _SegSumOp(_DO.DveOp):
    def compile(self, ver):
        key = (self.name, ver)
        if (r := _DO._COMPILE_CACHE.get(key)) is not None:
            return r
        result = _DveOpSpec(
            name=self.name,
            opcode=_DO.get_dve_sub_opcode(self.name),
            uops=_lower_segsum(self.spec, ver),
            rd1_en=True,
        )
        _DO._COMPILE_CACHE[key] = result
        return result


TT_SEGSUM_ANT = _SegSumOp(
    "TT_SEGSUM_ANT",
    _DS.Spec(body=_DS.scan(_DS.AluOp.ADD,
                           _DS.Bin(_DS.AluOp.MULTIPLY, _DS.Src0, _DS.Src1)),
             reference=_ref_segsum),
    subdim=True,
    uops_sha={},
)

if TT_SEGSUM_ANT.name not in _DO._SUB_OPCODE_FOR_NAME:
    _row = max(_DO._SUB_OPCODE_FOR_NAME.values()) + 1
    assert _row < 0x20
    _DO._SUB_OPCODE_FOR_NAME[TT_SEGSUM_ANT.name] = _row
    _DO.OPS.append(TT_SEGSUM_ANT)
    _DO.CUSTOM_DVE_SPECS[TT_SEGSUM_ANT.name] = TT_SEGSUM_ANT.spec


def _segsum(nc, out, in0, in1):
    return nc.vector._custom_dve(TT_SEGSUM_ANT, out=out, in0=in0, in1=in1)


# ---- problem constants (hardcoded per contract) ----------------------------
N_FULL = 20000
K = 16
MULT, NL, DIM = 8, 2, 4
EDGE_DIM, HID, FLAT = 32, 64, 256
NHEADS = 4
HEAD = MULT * DIM // NHEADS  # 8
SCALE = HEAD ** -0.5
INDICES = [0, 1, 1, 1]  # irrep index per spherical component

NCORES = 8
N_PAD = 20480            # padded node count (8 * 2560)
N_CORE = N_PAD // NCORES  # 2560 nodes per core
TILE_NODES = 128          # nodes per device tile
TILE_EDGES = TILE_NODES * K  # 2048 edges per tile
NCHUNK = TILE_EDGES // 128   # 16 chunks of 128 edges

F32 = mybir.dt.float32
I32 = mybir.dt.int32


def build_nc(n_core=N_CORE, n_gather=N_PAD, with_b2=False, debug=False):
    """Build the single-core Bass module (SPMD: same program on all cores)."""
    nt = n_core // TILE_NODES           # edge tiles per core
    assert n_core % TILE_NODES == 0
    nqc = (n_core + 511) // 512         # q/o GEMM chunks

    nc = bacc.Bacc(None, target_bir_lowering=False, debug=debug)
    ctx = ExitStack()
    names = {}

    with ExitStack() as bctx:
        tc = bctx.enter_context(tile.TileContext(nc))
        dram = bctx.enter_context(tc.tile_pool(name="dram", bufs=1, space="DRAM"))

        def din(name, shape, dt=F32):
            t = dram.tile(shape, dt, kind="ExternalInput", name=name,
                          uniquify=False)
            names[name] = name
            return t

        xT_d = din("xT", [EDGE_DIM, n_core * K])
        bk1_d = din("bk1", [nt, 128, NCHUNK, 64])
        bk2_d = din("bk2", [nt, 128, NCHUNK, 8])
        bv1_d = din("bv1", [nt, 128, NCHUNK, 64])
        bv2_d = din("bv2", [nt, 128, NCHUNK, 64])
        idx_d = din("idx", [nt, 128, NCHUNK], I32)
        fT_d = din("fT", [MULT * DIM, n_core])
        fall_d = din("fall", [n_gather, MULT * DIM])
        w1_d = din("w1", [EDGE_DIM, 2 * HID])
        w2_d = din("w2", [2 * HID, FLAT])
        b1_d = din("b1", [2 * HID, 1])
        b2k_d = din("b2k", [1, FLAT])
        b2v_d = din("b2v", [1, FLAT])
        Q2_d = din("Q2", [32, 32])
        qb_d = din("qb", [32, 1])
        O2_d = din("O2", [32, 32])
        ob_d = din("ob", [32, 1])
        amask_d = din("amask", [128, 8])
        out_d = dram.tile([32, n_core], F32, kind="ExternalOutput", name="outT",
                          uniquify=False)

        # ---- constant pool ------------------------------------------------
        cpool = bctx.enter_context(tc.tile_pool(name="const", bufs=1))
        w1_s = cpool.tile([EDGE_DIM, 2 * HID], F32)
        w2_s = cpool.tile([2 * HID, FLAT], F32)
        b1_s = cpool.tile([2 * HID, 1], F32)
        Q2_s = cpool.tile([32, 32], F32)
        qb_s = cpool.tile([32, 1], F32)
        O2_s = cpool.tile([32, 32], F32)
        ob_s = cpool.tile([32, 1], F32)
        amask_s = cpool.tile([128, 8], F32)
        ident_s = cpool.tile([128, 128], F32)
        fT_s = cpool.tile([EDGE_DIM, n_core], F32)
        q_nm = cpool.tile([128, nt * 32], F32)   # node-major q, [node%128, (tile, od)]
        ao_s = cpool.tile([32, n_core], F32)     # attention output (feature-major)
        if with_b2:
            b2k_s = cpool.tile([1, FLAT], F32)
            b2v_s = cpool.tile([1, FLAT], F32)
            ones_s = cpool.tile([1, 128], F32)

        nc.sync.dma_start(w1_s[:], w1_d[:])
        nc.sync.dma_start(w2_s[:], w2_d[:])
        nc.sync.dma_start(b1_s[:], b1_d[:])
        nc.sync.dma_start(Q2_s[:], Q2_d[:])
        nc.sync.dma_start(qb_s[:], qb_d[:])
        nc.sync.dma_start(O2_s[:], O2_d[:])
        nc.sync.dma_start(ob_s[:], ob_d[:])
        nc.sync.dma_start(amask_s[:], amask_d[:])
        nc.sync.dma_start(fT_s[:], fT_d[:])
        make_identity(nc, ident_s[:])
        if with_b2:
            nc.sync.dma_start(b2k_s[:], b2k_d[:])
            nc.sync.dma_start(b2v_s[:], b2v_d[:])
            nc.gpsimd.memset(ones_s[:], 1.0)

        # ---- pools (PSUM budget: 2+2+2+1+1 = 8 banks) ---------------------
        qsb = bctx.enter_context(tc.tile_pool(name="qsb", bufs=2))
        sb = bctx.enter_context(tc.tile_pool(name="sb", bufs=2))
        gsb = bctx.enter_context(tc.tile_pool(name="gsb", bufs=2))
        hps = bctx.enter_context(tc.tile_pool(name="hps", bufs=2, space="PSUM"))
        rwkp = bctx.enter_context(tc.tile_pool(name="rwkp", bufs=2, space="PSUM"))
        rwvp = bctx.enter_context(tc.tile_pool(name="rwvp", bufs=2, space="PSUM"))
        dpp = bctx.enter_context(tc.tile_pool(name="dpp", bufs=1, space="PSUM"))
        npp = bctx.enter_context(tc.tile_pool(name="npp", bufs=1, space="PSUM"))

        # ---- phase 1: q = eq_linear(f) node-major -------------------------
        for i in range(nqc):
            lo = i * 512
            hi = min(lo + 512, n_core)
            qp = hps.tile([128, 512], F32, tag="hp")
            nc.tensor.matmul(out=qp[:32, : hi - lo], lhsT=Q2_s[:], rhs=fT_s[:, lo:hi],
                             start=True, stop=True)
            qT = qsb.tile([32, 512], F32, tag="qT")
            nc.vector.tensor_scalar(out=qT[:, : hi - lo], in0=qp[:32, : hi - lo],
                                    scalar1=qb_s[:], scalar2=None,
                                    op0=mybir.AluOpType.add)
            # transpose 128-node blocks to node-major
            for b in range(lo // 128, hi // 128):
                qtp = npp.tile([128, 32], F32, tag="nps")
                nc.tensor.transpose(out=qtp[:], in_=qT[:, b * 128 - lo:(b + 1) * 128 - lo],
                                    identity=ident_s[:32, :32])
                nc.scalar.copy(out=q_nm[:, b * 32:(b + 1) * 32], in_=qtp[:])

        # ---- phase 2: edge tiles -----------------------------------------
        for t in range(nt):
            e0 = t * TILE_EDGES
            # --- loads ---
            xT_t = sb.tile([EDGE_DIM, TILE_EDGES], F32, tag="xT")
            nc.sync.dma_start(xT_t[:], xT_d[:, e0:e0 + TILE_EDGES])
            bk1 = sb.tile([128, NCHUNK * 64], F32, tag="bk1")
            nc.sync.dma_start(bk1[:], bk1_d[t])
            bk2 = sb.tile([128, NCHUNK * 8], F32, tag="bk2")
            nc.sync.dma_start(bk2[:], bk2_d[t])
            bv1 = sb.tile([128, NCHUNK * 64], F32, tag="bv1")
            nc.sync.dma_start(bv1[:], bv1_d[t])
            bv2 = sb.tile([128, NCHUNK * 64], F32, tag="bv2")
            nc.sync.dma_start(bv2[:], bv2_d[t])
            idx_t = sb.tile([128, NCHUNK], I32, tag="idx")
            nc.sync.dma_start(idx_t[:], idx_d[t])
            # q broadcast to edges: qe[p=(n*16+k), (c, od)] = q_nm[c*8+n, (t, od)]
            qe = sb.tile([128, NCHUNK * 32], F32, tag="qe")
            for c in range(NCHUNK):
                nc.sync.dma_start(
                    out=qe[:, c * 32:(c + 1) * 32],
                    in_=q_nm[c * 8:(c + 1) * 8, t * 32:(t + 1) * 32]
                        .unsqueeze(1).broadcast_to([8, K, 32]))
            # --- gather f_src ---
            fsrc = gsb.tile([128, NCHUNK * 32], F32, tag="fsrc")
            for c in range(NCHUNK):
                nc.gpsimd.indirect_dma_start(
                    out=fsrc[:, c * 32:(c + 1) * 32], out_offset=None,
                    in_=fall_d[:],
                    in_offset=bass.IndirectOffsetOnAxis(ap=idx_t[:, c:c + 1], axis=0),
                )
            # --- radial MLP layer 1 + gelu ---
            hT = sb.tile([128, TILE_EDGES], F32, tag="hT")
            for i in range(TILE_EDGES // 512):
                hp = hps.tile([128, 512], F32, tag="hp")
                nc.tensor.matmul(out=hp[:], lhsT=w1_s[:],
                                 rhs=xT_t[:, i * 512:(i + 1) * 512],
                                 start=True, stop=True)
                nc.scalar.activation(out=hT[:, i * 512:(i + 1) * 512], in_=hp[:],
                                     func=mybir.ActivationFunctionType.Gelu,
                                     bias=b1_s[:], scale=1.0)
            # --- tmp = f_src x bas1 (per conv) ---
            tmps = {}
            for cv, b1t in (("k", bk1), ("v", bv1)):
                # bas1 arrives m-replicated: [128, (c, m, l, d)].  Fused
                # multiply + d-segment scan; tmp = scan column d=3.
                tm = sb.tile([128, NCHUNK * 16], F32, tag=f"tmp{cv}")
                for l in range(2):
                    _segsum(
                        nc,
                        out=tm[:].rearrange("p (x l) -> p x l", l=2)[:, :, l]
                            .unsqueeze(2).broadcast_to([128, NCHUNK * 8, 4]),
                        in0=fsrc[:].rearrange("p (x d) -> p x d", d=4),
                        in1=b1t[:].rearrange("p (x l d) -> p x l d",
                                             l=2, d=4)[:, :, l, :])
                tmps[cv] = tm

            # --- radial layer 2 + per-edge t2 = rw @ tmp (fused seg-scan) ---
            t2s = {}
            for cv, lo_p, rwp, b2s_name in (("k", 0, rwkp, "b2k"),
                                            ("v", 64, rwvp, "b2v")):
                tm = tmps[cv]
                t2c = sb.tile([128, NCHUNK * 16], F32, tag=f"t2{cv}")
                for pair in range(NCHUNK // 2):
                    rw = rwp.tile([128, 512], F32, tag=f"rw{cv}")
                    for half in range(2):
                        ch = pair * 2 + half
                        mm_kw = dict(start=True, stop=(not with_b2))
                        nc.tensor.matmul(
                            out=rw[:, half * 256:(half + 1) * 256],
                            lhsT=hT[lo_p:lo_p + 64, ch * 128:(ch + 1) * 128],
                            rhs=w2_s[lo_p:lo_p + 64, :], **mm_kw)
                        if with_b2:
                            b2s = b2k_s if cv == "k" else b2v_s
                            nc.tensor.matmul(
                                out=rw[:, half * 256:(half + 1) * 256],
                                lhsT=ones_s[:], rhs=b2s[:],
                                start=False, stop=True)
                        _segsum(
                            nc,
                            out=t2c[:, ch * 16:(ch + 1) * 16]
                                .unsqueeze(2).broadcast_to([128, 16, 16]),
                            in0=rw[:, half * 256:(half + 1) * 256]
                                .rearrange("p (s n) -> p s n", s=16, n=16),
                            in1=tm[:, ch * 16:(ch + 1) * 16]
                                .unsqueeze(1).broadcast_to([128, 16, 16]))
                t2s[cv] = t2c

            # --- msg = t2 x bas2 ---
            msgs = {}
            # v-conv: fused per-d seg-scan over l (bas2v arrives o-replicated
            # in (o, d, l) order)
            vm = sb.tile([128, NCHUNK * 32], F32, tag="msgv")
            t2v_v = t2s["v"][:].rearrange("p (x l) -> p x l", l=2)
            bv2_v = bv2[:].rearrange("p (x d l) -> p x d l", d=4, l=2)
            for d in range(4):
                _segsum(
                    nc,
                    out=vm[:].rearrange("p (x d) -> p x d", d=4)[:, :, d]
                        .unsqueeze(2).broadcast_to([128, NCHUNK * 8, 2]),
                    in0=t2v_v,
                    in1=bv2_v[:, :, d, :])
            msgs["v"] = vm
            for cv, b2t in (("k", bk2),):
                t2 = t2s[cv]
                pr3 = sb.tile([128, NCHUNK * 64], F32, tag="msg_pr")
                pr3_v = pr3[:].rearrange("p (c o d l) -> p c o d l",
                                         c=NCHUNK, o=8, d=4, l=2)
                t2_v = t2[:].rearrange("p (c o l) -> p c o l",
                                       c=NCHUNK, o=8, l=2)
                # bas2 stored (l2, d): addr = l*4 + d
                b2_v = b2t[:].rearrange("p (c l d) -> p c d l",
                                        c=NCHUNK, l=2, d=4)
                for l in range(2):
                    nc.vector.tensor_tensor(
                        out=pr3_v[:, :, :, :, l],
                        in0=t2_v[:, :, :, l:l + 1].broadcast_to(
                            [128, NCHUNK, 8, 4]),
                        in1=b2_v[:, :, :, l:l + 1]
                            .rearrange("p c d l -> p c l d")
                            .broadcast_to([128, NCHUNK, 8, 4]),
                        op=mybir.AluOpType.mult)
                mg = sb.tile([128, NCHUNK * 32], F32, tag=f"msg{cv}")
                nc.vector.tensor_reduce(
                    out=mg[:],
                    in_=pr3[:].rearrange("p (x l) -> p x l", l=2),
                    axis=mybir.AxisListType.X, op=mybir.AluOpType.add)
                msgs[cv] = mg

            # --- attention scores + softmax (denom via mask-matmul) ---
            ssc = sb.tile([128, NCHUNK * 4], F32, tag="ssc")
            _segsum(nc,
                    out=ssc[:].unsqueeze(2).broadcast_to([128, NCHUNK * 4, 8]),
                    in0=qe[:].rearrange("p (s n) -> p s n", s=NCHUNK * 4, n=8),
                    in1=msgs["k"][:].rearrange("p (s n) -> p s n",
                                               s=NCHUNK * 4, n=8))
            Ee = sb.tile([128, NCHUNK * 4], F32, tag="Ee")
            nc.scalar.activation(
                out=Ee[:], in_=ssc[:],
                func=mybir.ActivationFunctionType.Exp, scale=float(SCALE))
            dps = dpp.tile([8, NCHUNK * 4], F32, tag="dps")
            for c in range(NCHUNK):
                nc.tensor.matmul(out=dps[:, c * 4:(c + 1) * 4], lhsT=amask_s[:],
                                 rhs=Ee[:, c * 4:(c + 1) * 4], start=True, stop=True)
            rds = sb.tile([8, NCHUNK * 4], F32, tag="rds")
            nc.vector.reciprocal(out=rds[:], in_=dps[:])
            re = sb.tile([128, NCHUNK * 4], F32, tag="re")
            for c in range(NCHUNK):
                nc.sync.dma_start(
                    out=re[:, c * 4:(c + 1) * 4],
                    in_=rds[:, c * 4:(c + 1) * 4]
                        .unsqueeze(1).broadcast_to([8, K, 4]))
            at = sb.tile([128, NCHUNK * 4], F32, tag="at")
            nc.vector.tensor_tensor(out=at[:], in0=Ee[:], in1=re[:],
                                    op=mybir.AluOpType.mult)
            # --- weighted v-messages + attention apply (mask-matmul) ---
            wv = sb.tile([128, NCHUNK * 32], F32, tag="wv")
            wv_v = wv[:].rearrange("p (c h o d) -> p c h o d",
                                   c=NCHUNK, h=4, o=2, d=4)
            vm_v = msgs["v"][:].rearrange("p (c h o d) -> p c h o d",
                                          c=NCHUNK, h=4, o=2, d=4)
            at_v = (at[:].rearrange("p (c h) -> p c h", c=NCHUNK, h=4)
                    .unsqueeze(3).broadcast_to([128, NCHUNK, 4, 4]))
            for o in range(2):
                nc.vector.tensor_tensor(
                    out=wv_v[:, :, :, o, :], in0=vm_v[:, :, :, o, :],
                    in1=at_v, op=mybir.AluOpType.mult)
            nps = npp.tile([32, NCHUNK * 8], F32, tag="nps")
            for c in range(NCHUNK):
                nc.tensor.matmul(out=nps[:, c * 8:(c + 1) * 8],
                                 lhsT=wv[:, c * 32:(c + 1) * 32],
                                 rhs=amask_s[:], start=True, stop=True)
            nc.scalar.copy(out=ao_s[:, t * 128:(t + 1) * 128], in_=nps[:])

        # ---- phase 3: output eq_linear -----------------------------------
        for i in range(nqc):
            lo = i * 512
            hi = min(lo + 512, n_core)
            op = hps.tile([128, 512], F32, tag="hp")
            nc.tensor.matmul(out=op[:32, : hi - lo], lhsT=O2_s[:],
                             rhs=ao_s[:, lo:hi], start=True, stop=True)
            oT = qsb.tile([32, 512], F32, tag="qT")
            nc.vector.tensor_scalar(out=oT[:, : hi - lo], in0=op[:32, : hi - lo],
                                    scalar1=ob_s[:], scalar2=None,
                                    op0=mybir.AluOpType.add)
            nc.sync.dma_start(out_d[:, lo:hi], oT[:, : hi - lo])

    nc.compile()
    return nc, names


# ---------------------------------------------------------------------------
def prep_inputs(inputs, n_core=N_CORE, n_pad=N_PAD, ncores=NCORES):
    """Host-side shard + relayout. Pure numpy layout work, no math."""
    def npf(x):
        return np.ascontiguousarray(np.asarray(x), dtype=np.float32)

    f = npf(inputs["f"])
    n_real = f.shape[0]
    idx = np.asarray(inputs["neighbor_idx"]).astype(np.int32)
    ef = npf(inputs["edge_feats"])
    bk1 = npf(inputs["basis_k1"])
    bk2 = npf(inputs["basis_k2"])
    bv1 = npf(inputs["basis_v1"])
    bv2 = npf(inputs["basis_v2"])

    def padn(a):
        if a.shape[0] == n_pad:
            return a
        pad = [(0, n_pad - a.shape[0])] + [(0, 0)] * (a.ndim - 1)
        return np.pad(a, pad)

    f_p, idx_p, ef_p = padn(f), padn(idx), padn(ef)
    bk1_p, bk2_p, bv1_p, bv2_p = padn(bk1), padn(bk2), padn(bv1), padn(bv2)

    nt = n_core // TILE_NODES

    def edge_tiles(a_feat8):
        # (n_core*K, 8) -> [nt, 128, NCHUNK, 8]
        return np.ascontiguousarray(
            a_feat8.reshape(nt, NCHUNK, 128, 8).transpose(0, 2, 1, 3))

    def edge_tiles64(a_feat8):
        # m-replicate (n_core*K, 8) -> [nt, 128, NCHUNK, 64]
        a64 = np.broadcast_to(a_feat8[:, None, :],
                              (a_feat8.shape[0], 8, 8)).reshape(-1, 64)
        return np.ascontiguousarray(
            a64.reshape(nt, NCHUNK, 128, 64).transpose(0, 2, 1, 3))

    k_w1, v_w1 = npf(inputs["k_w1"]), npf(inputs["v_w1"])
    k_w2, v_w2 = npf(inputs["k_w2"]), npf(inputs["v_w2"])
    k_b1, v_b1 = npf(inputs["k_b1"]), npf(inputs["v_b1"])
    k_b2, v_b2 = npf(inputs["k_b2"]), npf(inputs["v_b2"])
    q_w, q_b = npf(inputs["q_w"]), npf(inputs["q_b"])
    o_w, o_b = npf(inputs["o_w"]), npf(inputs["o_b"])

    w1c = np.ascontiguousarray(np.concatenate([k_w1, v_w1], axis=1))
    w2c = np.ascontiguousarray(np.concatenate([k_w2, v_w2], axis=0))
    b1c = np.concatenate([k_b1, v_b1]).reshape(2 * HID, 1)

    def eq_mat(w):
        m = np.zeros((32, 32), np.float32)
        for d in range(DIM):
            l = INDICES[d]
            for m2 in range(MULT):
                for mm in range(MULT):
                    m[mm * DIM + d, m2 * DIM + d] = w[l * MULT + m2, mm]
        return m

    Q2, O2 = eq_mat(q_w), eq_mat(o_w)
    qb = np.zeros((32, 1), np.float32)
    qb[::DIM, 0] = q_b[:, 0]
    ob = np.zeros((32, 1), np.float32)
    ob[::DIM, 0] = o_b[:, 0]
    amask = np.zeros((128, 8), np.float32)
    amask[np.arange(128), np.arange(128) // K] = 1.0

    fall = np.ascontiguousarray(f_p.reshape(n_pad, 32))
    with_b2 = bool(np.any(k_b2) or np.any(v_b2))

    in_maps = []
    for c in range(ncores):
        lo, hi = c * n_core, (c + 1) * n_core
        E = n_core * K
        m = {
            "xT": np.ascontiguousarray(ef_p[lo:hi].reshape(E, 32).T),
            "bk1": edge_tiles64(bk1_p[lo:hi].transpose(0, 1, 3, 2).reshape(E, 8)),
            "bk2": edge_tiles(bk2_p[lo:hi].reshape(E, 8)),
            "bv1": edge_tiles64(bv1_p[lo:hi].transpose(0, 1, 3, 2).reshape(E, 8)),
            "bv2": edge_tiles64(
                bv2_p[lo:hi].reshape(E, 2, 4).transpose(0, 2, 1).reshape(E, 8)),
            "idx": np.ascontiguousarray(
                idx_p[lo:hi].reshape(nt, NCHUNK, 128).transpose(0, 2, 1)),
            "fT": np.ascontiguousarray(f_p[lo:hi].reshape(n_core, 32).T),
            "fall": fall,
            "w1": w1c, "w2": w2c, "b1": b1c,
            "b2k": k_b2.reshape(1, FLAT), "b2v": v_b2.reshape(1, FLAT),
            "Q2": Q2, "qb": qb, "O2": O2, "ob": ob, "amask": amask,
        }
        in_maps.append(m)
    return in_maps, with_b2, n_real


def postprocess(results, n_real=N_FULL, n_core=N_CORE):
    parts = []
    for r in results:
        oT = r["outT"]  # [32, n_core]
        parts.append(oT.reshape(MULT, DIM, n_core).transpose(2, 0, 1))
    out = np.concatenate(parts, axis=0)[:n_real]
    return np.ascontiguousarray(out, dtype=np.float32)


_NC_CACHE = {}


def kernel(**inputs):
    from concourse.bass_utils import run_bass_kernel_spmd

    in_maps, with_b2, n_real = prep_inputs(inputs)
    key = ("full", with_b2)
    if key not in _NC_CACHE:
        _NC_CACHE[key] = build_nc(with_b2=with_b2)
    nc, _ = _NC_CACHE[key]
    res = run_bass_kernel_spmd(nc, in_maps, core_ids=list(range(NCORES)))
    return postprocess(res.results, n_real=n_real)


# revision 34
# speedup vs baseline: 1.0576x; 1.0576x over previous
"""Trainium2 Bass kernel for nn_EquivariantAttention (gnn_message_passing).

Sharding: nodes (and their 16 edge slots) are split across the 8 NeuronCores
(2560 padded nodes per core).  Node features `f` are replicated to every core
(2.5 MB) and the per-edge gather f[neighbor_idx] is done on-device with
indirect DMA.  Radial MLPs run on TensorE, per-edge tensor products on
VectorE, softmax denominators / attention-weighted sums via small mask
matmuls on TensorE (reducing over the 16 edge slots that live in the
partition dimension).

Self-contained: only needs numpy + the system toolchain at /opt/trn_rl_repo.
"""

import sys
from contextlib import ExitStack

import numpy as np

if "/opt/trn_rl_repo" not in sys.path:
    sys.path.insert(0, "/opt/trn_rl_repo")

import concourse.bacc as bacc
import concourse.mybir as mybir
import concourse.tile as tile
from concourse import bass
from concourse.masks import make_identity


# ---- custom DVE op: fused multiply + segmented running-sum ------------------
import dataclasses as _dc

from concourse import dve_spec as _DS
from concourse import dve_ops as _DO
from concourse.dve_uop import DveOpSpec as _DveOpSpec


def _ref_segsum(in0, in1, c0, c1, c2):
    x = in0.astype(np.float32) * in1.astype(np.float32)
    return np.cumsum(x, axis=-1)


def _lower_segsum(spec, ver):
    n_lanes, n_stages = _DS.N_LANES[ver], _DS.N_STAGES[ver]
    _DS._validate_body(spec, ver)
    spec2 = _DS._hoist_stream_invariant_ops(spec)
    scans = _DS._collect(spec2.body, _DS.Scan)
    latches = _DS._collect(spec2.body, _DS.Latch)
    p = _DS._build_placement(spec2, scans, n_stages, n_lanes)
    states = _DS._build_state_machine(spec2, scans, latches, p)
    sc = scans[0]
    d = p.node_stage[sc]
    step_ov = {d: _DS._Stage(_DS.AluOp.BYPASS, sc.expr)}
    seed, steady = states
    steady2 = _dc.replace(
        seed.__class__(**{**_dc.asdict(steady)}) if False else steady,
        trigger=(_DS.Trigger.SRC_TENSOR_DONE, _DS.Trigger.SUB_DIM_DONE,
                 _DS.Trigger.NONE),
        next=(0, 2, 0),
    )
    step = _dc.replace(
        steady,
        overrides=step_ov,
        trigger=(_DS.Trigger.SRC_TENSOR_DONE, _DS.Trigger.SUB_DIM_DONE,
                 _DS.Trigger.COUNT),
        next=(0, 2, 1),
        repeat=1,
    )
    uops = [_DS._assemble(s) for s in (seed, steady2, step)]
    for u in uops:
        u.validate(ver)
    return uops


@_dc.dataclass(frozen=True)
class _SegSumOp(_DO.DveOp):
    def compile(self, ver):
        key = (self.name, ver)
        if (r := _DO._COMPILE_CACHE.get(key)) is not None:
            return r
        result = _DveOpSpec(
            name=self.name,
            opcode=_DO.get_dve_sub_opcode(self.name),
            uops=_lower_segsum(self.spec, ver),
            rd1_en=True,
        )
        _DO._COMPILE_CACHE[key] = result
        return result


TT_SEGSUM_ANT = _SegSumOp(
    "TT_SEGSUM_ANT",
    _DS.Spec(body=_DS.scan(_DS.AluOp.ADD,
                           _DS.Bin(_DS.AluOp.MULTIPLY, _DS.Src0, _DS.Src1)),
             reference=_ref_segsum),
    subdim=True,
    uops_sha={},
)

if TT_SEGSUM_ANT.name not in _DO._SUB_OPCODE_FOR_NAME:
    _row = max(_DO._SUB_OPCODE_FOR_NAME.values()) + 1
    assert _row < 0x20
    _DO._SUB_OPCODE_FOR_NAME[TT_SEGSUM_ANT.name] = _row
    _DO.OPS.append(TT_SEGSUM_ANT)
    _DO.CUSTOM_DVE_SPECS[TT_SEGSUM_ANT.name] = TT_SEGSUM_ANT.spec


def _segsum(nc, out, in0, in1):
    return nc.vector._custom_dve(TT_SEGSUM_ANT, out=out, in0=in0, in1=in1)


# ---- problem constants (hardcoded per contract) ----------------------------
N_FULL = 20000
K = 16
MULT, NL, DIM = 8, 2, 4
EDGE_DIM, HID, FLAT = 32, 64, 256
NHEADS = 4
HEAD = MULT * DIM // NHEADS  # 8
SCALE = HEAD ** -0.5
INDICES = [0, 1, 1, 1]  # irrep index per spherical component

NCORES = 8
N_PAD = 20480            # padded node count (8 * 2560)
N_CORE = N_PAD // NCORES  # 2560 nodes per core
TILE_NODES = 128          # nodes per device tile
TILE_EDGES = TILE_NODES * K  # 2048 edges per tile
NCHUNK = TILE_EDGES // 128   # 16 chunks of 128 edges

F32 = mybir.dt.float32
I32 = mybir.dt.int32


def build_nc(n_core=N_CORE, n_gather=N_PAD, with_b2=False, debug=False):
    """Build the single-core Bass module (SPMD: same program on all cores)."""
    nt = n_core // TILE_NODES           # edge tiles per core
    assert n_core % TILE_NODES == 0
    nqc = (n_core + 511) // 512         # q/o GEMM chunks

    nc = bacc.Bacc(None, target_bir_lowering=False, debug=debug)
    ctx = ExitStack()
    names = {}

    with ExitStack() as bctx:
        tc = bctx.enter_context(tile.TileContext(nc))
        dram = bctx.enter_context(tc.tile_pool(name="dram", bufs=1, space="DRAM"))

        def din(name, shape, dt=F32):
            t = dram.tile(shape, dt, kind="ExternalInput", name=name,
                          uniquify=False)
            names[name] = name
            return t

        xT_d = din("xT", [EDGE_DIM, n_core * K])
        bk1_d = din("bk1", [nt, 128, NCHUNK, 8])
        bk2_d = din("bk2", [nt, 128, NCHUNK, 8])
        bv1_d = din("bv1", [nt, 128, NCHUNK, 8])
        bv2_d = din("bv2", [nt, 128, NCHUNK, 8])
        idx_d = din("idx", [nt, 128, NCHUNK], I32)
        fT_d = din("fT", [MULT * DIM, n_core])
        fall_d = din("fall", [n_gather, MULT * DIM])
        w1_d = din("w1", [EDGE_DIM, 2 * HID])
        w2_d = din("w2", [2 * HID, FLAT])
        b1_d = din("b1", [2 * HID, 1])
        b2k_d = din("b2k", [1, FLAT])
        b2v_d = din("b2v", [1, FLAT])
        Q2_d = din("Q2", [32, 32])
        qb_d = din("qb", [32, 1])
        O2_d = din("O2", [32, 32])
        ob_d = din("ob", [32, 1])
        amask_d = din("amask", [128, 8])
        out_d = dram.tile([32, n_core], F32, kind="ExternalOutput", name="outT",
                          uniquify=False)

        # ---- constant pool ------------------------------------------------
        cpool = bctx.enter_context(tc.tile_pool(name="const", bufs=1))
        w1_s = cpool.tile([EDGE_DIM, 2 * HID], F32)
        w2_s = cpool.tile([2 * HID, FLAT], F32)
        b1_s = cpool.tile([2 * HID, 1], F32)
        Q2_s = cpool.tile([32, 32], F32)
        qb_s = cpool.tile([32, 1], F32)
        O2_s = cpool.tile([32, 32], F32)
        ob_s = cpool.tile([32, 1], F32)
        amask_s = cpool.tile([128, 8], F32)
        ident_s = cpool.tile([128, 128], F32)
        fT_s = cpool.tile([EDGE_DIM, n_core], F32)
        q_nm = cpool.tile([128, nt * 32], F32)   # node-major q, [node%128, (tile, od)]
        ao_s = cpool.tile([32, n_core], F32)     # attention output (feature-major)
        if with_b2:
            b2k_s = cpool.tile([1, FLAT], F32)
            b2v_s = cpool.tile([1, FLAT], F32)
            ones_s = cpool.tile([1, 128], F32)

        nc.sync.dma_start(w1_s[:], w1_d[:])
        nc.sync.dma_start(w2_s[:], w2_d[:])
        nc.sync.dma_start(b1_s[:], b1_d[:])
        nc.sync.dma_start(Q2_s[:], Q2_d[:])
        nc.sync.dma_start(qb_s[:], qb_d[:])
        nc.sync.dma_start(O2_s[:], O2_d[:])
        nc.sync.dma_start(ob_s[:], ob_d[:])
        nc.sync.dma_start(amask_s[:], amask_d[:])
        nc.sync.dma_start(fT_s[:], fT_d[:])
        make_identity(nc, ident_s[:])
        if with_b2:
            nc.sync.dma_start(b2k_s[:], b2k_d[:])
            nc.sync.dma_start(b2v_s[:], b2v_d[:])
            nc.gpsimd.memset(ones_s[:], 1.0)

        # ---- pools (PSUM budget: 2+2+2+1+1 = 8 banks) ---------------------
        qsb = bctx.enter_context(tc.tile_pool(name="qsb", bufs=2))
        sb = bctx.enter_context(tc.tile_pool(name="sb", bufs=2))
        gsb = bctx.enter_context(tc.tile_pool(name="gsb", bufs=2))
        hps = bctx.enter_context(tc.tile_pool(name="hps", bufs=2, space="PSUM"))
        rwkp = bctx.enter_context(tc.tile_pool(name="rwkp", bufs=2, space="PSUM"))
        rwvp = bctx.enter_context(tc.tile_pool(name="rwvp", bufs=2, space="PSUM"))
        dpp = bctx.enter_context(tc.tile_pool(name="dpp", bufs=1, space="PSUM"))
        npp = bctx.enter_context(tc.tile_pool(name="npp", bufs=1, space="PSUM"))

        # ---- phase 1: q = eq_linear(f) node-major -------------------------
        for i in range(nqc):
            lo = i * 512
            hi = min(lo + 512, n_core)
            qp = hps.tile([128, 512], F32, tag="hp")
            nc.tensor.matmul(out=qp[:32, : hi - lo], lhsT=Q2_s[:], rhs=fT_s[:, lo:hi],
                             start=True, stop=True)
            qT = qsb.tile([32, 512], F32, tag="qT")
            nc.vector.tensor_scalar(out=qT[:, : hi - lo], in0=qp[:32, : hi - lo],
                                    scalar1=qb_s[:], scalar2=None,
                                    op0=mybir.AluOpType.add)
            # transpose 128-node blocks to node-major
            for b in range(lo // 128, hi // 128):
                qtp = npp.tile([128, 32], F32, tag="nps")
                nc.tensor.transpose(out=qtp[:], in_=qT[:, b * 128 - lo:(b + 1) * 128 - lo],
                                    identity=ident_s[:32, :32])
                nc.scalar.copy(out=q_nm[:, b * 32:(b + 1) * 32], in_=qtp[:])

        # ---- phase 2: edge tiles -----------------------------------------
        for t in range(nt):
            e0 = t * TILE_EDGES
            # --- loads ---
            xT_t = sb.tile([EDGE_DIM, TILE_EDGES], F32, tag="xT")
            nc.sync.dma_start(xT_t[:], xT_d[:, e0:e0 + TILE_EDGES])
            bk1c = sb.tile([128, NCHUNK * 8], F32, tag="bk1c")
            nc.sync.dma_start(bk1c[:], bk1_d[t])
            bk1 = sb.tile([128, NCHUNK * 64], F32, tag="bk1")
            nc.scalar.copy(
                out=bk1[:].rearrange("p (c m q) -> p c m q",
                                      c=NCHUNK, m=8, q=8),
                in_=bk1c[:].rearrange("p (c q) -> p c q", c=NCHUNK, q=8)
                    .unsqueeze(2).broadcast_to([128, NCHUNK, 8, 8]))
            bk2c = sb.tile([128, NCHUNK * 8], F32, tag="bk2c")
            nc.sync.dma_start(bk2c[:], bk2_d[t])
            bk2 = sb.tile([128, NCHUNK * 64], F32, tag="bk2")
            nc.scalar.copy(
                out=bk2[:].rearrange("p (c o q) -> p c o q",
                                     c=NCHUNK, o=8, q=8),
                in_=bk2c[:].rearrange("p (c q) -> p c q", c=NCHUNK, q=8)
                    .unsqueeze(2).broadcast_to([128, NCHUNK, 8, 8]))
            bv1c = sb.tile([128, NCHUNK * 8], F32, tag="bv1c")
            nc.sync.dma_start(bv1c[:], bv1_d[t])
            bv1 = sb.tile([128, NCHUNK * 64], F32, tag="bv1")
            nc.scalar.copy(
                out=bv1[:].rearrange("p (c m q) -> p c m q",
                                      c=NCHUNK, m=8, q=8),
                in_=bv1c[:].rearrange("p (c q) -> p c q", c=NCHUNK, q=8)
                    .unsqueeze(2).broadcast_to([128, NCHUNK, 8, 8]))
            bv2c = sb.tile([128, NCHUNK * 8], F32, tag="bv2c")
            nc.sync.dma_start(bv2c[:], bv2_d[t])
            bv2 = sb.tile([128, NCHUNK * 64], F32, tag="bv2")
            nc.scalar.copy(
                out=bv2[:].rearrange("p (c o q) -> p c o q",
                                     c=NCHUNK, o=8, q=8),
                in_=bv2c[:].rearrange("p (c q) -> p c q", c=NCHUNK, q=8)
                    .unsqueeze(2).broadcast_to([128, NCHUNK, 8, 8]))
            idx_t = sb.tile([128, NCHUNK], I32, tag="idx")
            nc.sync.dma_start(idx_t[:], idx_d[t])
            # q broadcast to edges: qe[p=(n*16+k), (c, od)] = q_nm[c*8+n, (t, od)]
            qe = sb.tile([128, NCHUNK * 32], F32, tag="qe")
            for c in range(NCHUNK):
                nc.sync.dma_start(
                    out=qe[:, c * 32:(c + 1) * 32],
                    in_=q_nm[c * 8:(c + 1) * 8, t * 32:(t + 1) * 32]
                        .unsqueeze(1).broadcast_to([8, K, 32]))
            # --- gather f_src ---
            fsrc = gsb.tile([128, NCHUNK * 32], F32, tag="fsrc")
            for c in range(NCHUNK):
                nc.gpsimd.indirect_dma_start(
                    out=fsrc[:, c * 32:(c + 1) * 32], out_offset=None,
                    in_=fall_d[:],
                    in_offset=bass.IndirectOffsetOnAxis(ap=idx_t[:, c:c + 1], axis=0),
                )
            # --- radial MLP layer 1 + gelu ---
            hT = sb.tile([128, TILE_EDGES], F32, tag="hT")
            for i in range(TILE_EDGES // 512):
                hp = hps.tile([128, 512], F32, tag="hp")
                nc.tensor.matmul(out=hp[:], lhsT=w1_s[:],
                                 rhs=xT_t[:, i * 512:(i + 1) * 512],
                                 start=True, stop=True)
                nc.scalar.activation(out=hT[:, i * 512:(i + 1) * 512], in_=hp[:],
                                     func=mybir.ActivationFunctionType.Gelu,
                                     bias=b1_s[:], scale=1.0)
            # --- tmp = f_src x bas1 (per conv) ---
            tmps = {}
            for cv, b1t in (("k", bk1), ("v", bv1)):
                # bas1 arrives m-replicated: [128, (c, m, l, d)].  Fused
                # multiply + d-segment scan; tmp = scan column d=3.
                tms = sb.tile([128, NCHUNK * 64], F32, tag=f"tmpsc{cv}")
                for l in range(2):
                    _segsum(
                        nc,
                        out=tms[:].rearrange("p (x l d) -> p x l d",
                                             l=2, d=4)[:, :, l, :],
                        in0=fsrc[:].rearrange("p (x d) -> p x d", d=4),
                        in1=b1t[:].rearrange("p (x l d) -> p x l d",
                                             l=2, d=4)[:, :, l, :])
                tmps[cv] = tms

            # --- radial layer 2 + per-edge t2 = rw @ tmp (fused seg-scan) ---
            t2s = {}
            for cv, lo_p, rwp, b2s_name in (("k", 0, rwkp, "b2k"),
                                            ("v", 64, rwvp, "b2v")):
                tm = tmps[cv]
                t2c = sb.tile([128, NCHUNK * 256], F32, tag=f"t2s{cv}")
                for pair in range(NCHUNK // 2):
                    rw = rwp.tile([128, 512], F32, tag=f"rw{cv}")
                    for half in range(2):
                        ch = pair * 2 + half
                        mm_kw = dict(start=True, stop=(not with_b2))
                        nc.tensor.matmul(
                            out=rw[:, half * 256:(half + 1) * 256],
                            lhsT=hT[lo_p:lo_p + 64, ch * 128:(ch + 1) * 128],
                            rhs=w2_s[lo_p:lo_p + 64, :], **mm_kw)
                        if with_b2:
                            b2s = b2k_s if cv == "k" else b2v_s
                            nc.tensor.matmul(
                                out=rw[:, half * 256:(half + 1) * 256],
                                lhsT=ones_s[:], rhs=b2s[:],
                                start=False, stop=True)
                        _segsum(
                            nc,
                            out=t2c[:, ch * 256:(ch + 1) * 256]
                                .rearrange("p (s n) -> p s n", s=16, n=16),
                            in0=rw[:, half * 256:(half + 1) * 256]
                                .rearrange("p (s n) -> p s n", s=16, n=16),
                            in1=tm[:, ch * 64:(ch + 1) * 64]
                                .rearrange("p (j d) -> p j d", d=4)[:, :, 3]
                                .unsqueeze(1).broadcast_to([128, 16, 16]))
                t2s[cv] = t2c

            # --- msg = t2 x bas2 (fused per-d seg-scan over l) ---
            msgs = {}
            for cv, b2t in (("k", bk2), ("v", bv2)):
                t2v = (t2s[cv][:]
                       .rearrange("p (x n) -> p x n", n=16)[:, :, 15]
                       .rearrange("p (x l) -> p x l", l=2))
                b2r = b2t[:].rearrange("p (x d l) -> p x d l", d=4, l=2)
                mg = sb.tile([128, NCHUNK * 32], F32, tag=f"msg{cv}")
                for d in range(4):
                    _segsum(
                        nc,
                        out=mg[:].rearrange("p (x d) -> p x d", d=4)[:, :, d]
                            .unsqueeze(2).broadcast_to([128, NCHUNK * 8, 2]),
                        in0=t2v,
                        in1=b2r[:, :, d, :])
                msgs[cv] = mg

            # --- attention scores + softmax (denom via mask-matmul) ---
            sp = sb.tile([128, NCHUNK * 32], F32, tag="sp")
            _segsum(nc,
                    out=sp[:].rearrange("p (s n) -> p s n", s=NCHUNK * 4, n=8),
                    in0=qe[:].rearrange("p (s n) -> p s n", s=NCHUNK * 4, n=8),
                    in1=msgs["k"][:].rearrange("p (s n) -> p s n",
                                               s=NCHUNK * 4, n=8))
            Ee = sb.tile([128, NCHUNK * 4], F32, tag="Ee")
            nc.scalar.activation(
                out=Ee[:],
                in_=sp[:].rearrange("p (s n) -> p s n", s=NCHUNK * 4, n=8)[:, :, 7],
                func=mybir.ActivationFunctionType.Exp, scale=float(SCALE))
            dps = dpp.tile([8, NCHUNK * 4], F32, tag="dps")
            for c in range(NCHUNK):
                nc.tensor.matmul(out=dps[:, c * 4:(c + 1) * 4], lhsT=amask_s[:],
                                 rhs=Ee[:, c * 4:(c + 1) * 4], start=True, stop=True)
            rds = sb.tile([8, NCHUNK * 4], F32, tag="rds")
            nc.vector.reciprocal(out=rds[:], in_=dps[:])
            re = sb.tile([128, NCHUNK * 4], F32, tag="re")
            for c in range(NCHUNK):
                nc.sync.dma_start(
                    out=re[:, c * 4:(c + 1) * 4],
                    in_=rds[:, c * 4:(c + 1) * 4]
                        .unsqueeze(1).broadcast_to([8, K, 4]))
            at = sb.tile([128, NCHUNK * 4], F32, tag="at")
            nc.vector.tensor_tensor(out=at[:], in0=Ee[:], in1=re[:],
                                    op=mybir.AluOpType.mult)
            # --- weighted v-messages + attention apply (mask-matmul) ---
            wv = sb.tile([128, NCHUNK * 32], F32, tag="wv")
            wv_v = wv[:].rearrange("p (c h o d) -> p c h o d",
                                   c=NCHUNK, h=4, o=2, d=4)
            vm_v = msgs["v"][:].rearrange("p (c h o d) -> p c h o d",
                                          c=NCHUNK, h=4, o=2, d=4)
            at_v = (at[:].rearrange("p (c h) -> p c h", c=NCHUNK, h=4)
                    .unsqueeze(3).broadcast_to([128, NCHUNK, 4, 4]))
            for o in range(2):
                nc.vector.tensor_tensor(
                    out=wv_v[:, :, :, o, :], in0=vm_v[:, :, :, o, :],
                    in1=at_v, op=mybir.AluOpType.mult)
            nps = npp.tile([32, NCHUNK * 8], F32, tag="nps")
            for c in range(NCHUNK):
                nc.tensor.matmul(out=nps[:, c * 8:(c + 1) * 8],
                                 lhsT=wv[:, c * 32:(c + 1) * 32],
                                 rhs=amask_s[:], start=True, stop=True)
            nc.scalar.copy(out=ao_s[:, t * 128:(t + 1) * 128], in_=nps[:])

        # ---- phase 3: output eq_linear -----------------------------------
        for i in range(nqc):
            lo = i * 512
            hi = min(lo + 512, n_core)
            op = hps.tile([128, 512], F32, tag="hp")
            nc.tensor.matmul(out=op[:32, : hi - lo], lhsT=O2_s[:],
                             rhs=ao_s[:, lo:hi], start=True, stop=True)
            oT = qsb.tile([32, 512], F32, tag="qT")
            nc.vector.tensor_scalar(out=oT[:, : hi - lo], in0=op[:32, : hi - lo],
                                    scalar1=ob_s[:], scalar2=None,
                                    op0=mybir.AluOpType.add)
            nc.sync.dma_start(out_d[:, lo:hi], oT[:, : hi - lo])

    nc.compile()
    return nc, names


# ---------------------------------------------------------------------------
def prep_inputs(inputs, n_core=N_CORE, n_pad=N_PAD, ncores=NCORES):
    """Host-side shard + relayout. Pure numpy layout work, no math."""
    def npf(x):
        return np.ascontiguousarray(np.asarray(x), dtype=np.float32)

    f = npf(inputs["f"])
    n_real = f.shape[0]
    idx = np.asarray(inputs["neighbor_idx"]).astype(np.int32)
    ef = npf(inputs["edge_feats"])
    bk1 = npf(inputs["basis_k1"])
    bk2 = npf(inputs["basis_k2"])
    bv1 = npf(inputs["basis_v1"])
    bv2 = npf(inputs["basis_v2"])

    def padn(a):
        if a.shape[0] == n_pad:
            return a
        pad = [(0, n_pad - a.shape[0])] + [(0, 0)] * (a.ndim - 1)
        return np.pad(a, pad)

    f_p, idx_p, ef_p = padn(f), padn(idx), padn(ef)
    bk1_p, bk2_p, bv1_p, bv2_p = padn(bk1), padn(bk2), padn(bv1), padn(bv2)

    nt = n_core // TILE_NODES

    def edge_tiles(a_feat8):
        # (n_core*K, 8) -> [nt, 128, NCHUNK, 8]
        return np.ascontiguousarray(
            a_feat8.reshape(nt, NCHUNK, 128, 8).transpose(0, 2, 1, 3))

    def edge_tiles64(a_feat8):
        # m-replicate (n_core*K, 8) -> [nt, 128, NCHUNK, 64]
        a64 = np.broadcast_to(a_feat8[:, None, :],
                              (a_feat8.shape[0], 8, 8)).reshape(-1, 64)
        return np.ascontiguousarray(
            a64.reshape(nt, NCHUNK, 128, 64).transpose(0, 2, 1, 3))

    k_w1, v_w1 = npf(inputs["k_w1"]), npf(inputs["v_w1"])
    k_w2, v_w2 = npf(inputs["k_w2"]), npf(inputs["v_w2"])
    k_b1, v_b1 = npf(inputs["k_b1"]), npf(inputs["v_b1"])
    k_b2, v_b2 = npf(inputs["k_b2"]), npf(inputs["v_b2"])
    q_w, q_b = npf(inputs["q_w"]), npf(inputs["q_b"])
    o_w, o_b = npf(inputs["o_w"]), npf(inputs["o_b"])

    w1c = np.ascontiguousarray(np.concatenate([k_w1, v_w1], axis=1))
    w2c = np.ascontiguousarray(np.concatenate([k_w2, v_w2], axis=0))
    b1c = np.concatenate([k_b1, v_b1]).reshape(2 * HID, 1)

    def eq_mat(w):
        m = np.zeros((32, 32), np.float32)
        for d in range(DIM):
            l = INDICES[d]
            for m2 in range(MULT):
                for mm in range(MULT):
                    m[mm * DIM + d, m2 * DIM + d] = w[l * MULT + m2, mm]
        return m

    Q2, O2 = eq_mat(q_w), eq_mat(o_w)
    qb = np.zeros((32, 1), np.float32)
    qb[::DIM, 0] = q_b[:, 0]
    ob = np.zeros((32, 1), np.float32)
    ob[::DIM, 0] = o_b[:, 0]
    amask = np.zeros((128, 8), np.float32)
    amask[np.arange(128), np.arange(128) // K] = 1.0

    fall = np.ascontiguousarray(f_p.reshape(n_pad, 32))
    with_b2 = bool(np.any(k_b2) or np.any(v_b2))

    in_maps = []
    for c in range(ncores):
        lo, hi = c * n_core, (c + 1) * n_core
        E = n_core * K
        m = {
            "xT": np.ascontiguousarray(ef_p[lo:hi].reshape(E, 32).T),
            "bk1": edge_tiles(bk1_p[lo:hi].transpose(0, 1, 3, 2).reshape(E, 8)),
            "bk2": edge_tiles(
                bk2_p[lo:hi].reshape(E, 2, 4).transpose(0, 2, 1).reshape(E, 8)),
            "bv1": edge_tiles(bv1_p[lo:hi].transpose(0, 1, 3, 2).reshape(E, 8)),
            "bv2": edge_tiles(
                bv2_p[lo:hi].reshape(E, 2, 4).transpose(0, 2, 1).reshape(E, 8)),
            "idx": np.ascontiguousarray(
                idx_p[lo:hi].reshape(nt, NCHUNK, 128).transpose(0, 2, 1)),
            "fT": np.ascontiguousarray(f_p[lo:hi].reshape(n_core, 32).T),
            "fall": fall,
            "w1": w1c, "w2": w2c, "b1": b1c,
            "b2k": k_b2.reshape(1, FLAT), "b2v": v_b2.reshape(1, FLAT),
            "Q2": Q2, "qb": qb, "O2": O2, "ob": ob, "amask": amask,
        }
        in_maps.append(m)
    return in_maps, with_b2, n_real


def postprocess(results, n_real=N_FULL, n_core=N_CORE):
    parts = []
    for r in results:
        oT = r["outT"]  # [32, n_core]
        parts.append(oT.reshape(MULT, DIM, n_core).transpose(2, 0, 1))
    out = np.concatenate(parts, axis=0)[:n_real]
    return np.ascontiguousarray(out, dtype=np.float32)


_NC_CACHE = {}


def kernel(**inputs):
    from concourse.bass_utils import run_bass_kernel_spmd

    in_maps, with_b2, n_real = prep_inputs(inputs)
    key = ("full", with_b2)
    if key not in _NC_CACHE:
        _NC_CACHE[key] = build_nc(with_b2=with_b2)
    nc, _ = _NC_CACHE[key]
    res = run_bass_kernel_spmd(nc, in_maps, core_ids=list(range(NCORES)))
    return postprocess(res.results, n_real=n_real)
